# Initial kernel scaffold
#
"""PointNet++ (SA x3 + FP x3 + head) on 8 Trainium2 NeuronCores via Bass/Tile.

Strategy (v2: morton pair-window gathers)
-----------------------------------------
- Host: FPS / radius-kNN / kNN-interp selection (pos-derived only), morton
  sort of every level's points, pair-window covers of each center's exact
  neighbor set, BN/bias folding, index wrapping.
- Device (per core, channel-major): as before, but the SA neighbor gather
  uses a "pair stripe" z table: stripe 2j = z rows (j, j+1) packed compact
  (768B: zA_j | zA_{j+1} | zB_j zB_{j+1} pad), stripe 2j+1 = (j, j).
  Morton ordering makes neighbor sets contiguous runs, so ~2 slots ride on
  every gather index (the SWDGE gather costs ~7.7ns/index regardless of
  element bytes up to ~1KB). Odd runs overlap-duplicate; isolated rows use
  the self stripe; duplicates never change a max.
- Sharding: cores 2b and 2b+1 both compute cloud b; host takes even cores.
"""

import numpy as np
import ml_dtypes

import jax
import jax.numpy as jnp
from jax import lax

import concourse.bass as bass
import concourse.mybir as mybir
from concourse import bacc
import concourse.tile as tile
from concourse.bass_utils import run_bass_kernel_spmd

# ---------------------------------------------------------------- constants
B, N0, NIN, NH, NOUT = 4, 4096, 16, 128, 2
DEPTH, K, RADIUS, KNN, EPS = 3, 64, 2.0, 3, 1e-5
D = NH + 3        # 131, PointConv MLP width
BNS = np.float32(1.0 / np.sqrt(1.0 + EPS))
BF16 = mybir.dt.bfloat16
F32 = mybir.dt.float32
I16 = mybir.dt.int16
bf16 = ml_dtypes.bfloat16

LEVELS = [N0 // (2 ** i) for i in range(DEPTH + 1)]   # [4096, 2048, 1024, 512]
CC = 16           # centers per sub-chunk (2 sub-chunks per gather)
STRIPE = 384      # bf16 elems per pair stripe (768B)

_cpu = jax.devices("cpu")[0]

# ---------------------------------------------------------------- host prep

def _fps_ref(pts, m):
    n = pts.shape[0]

    def step(carry, _):
        dmin, last = carry
        d = jnp.sum((pts - pts[last]) ** 2, axis=-1)
        dmin = jnp.minimum(dmin, d)
        nxt = jnp.argmax(dmin).astype(jnp.int32)
        return (dmin, nxt), nxt

    _, rest = lax.scan(step, (jnp.full((n,), 1e30, pts.dtype), jnp.int32(0)),
                       None, length=m - 1)
    return jnp.concatenate([jnp.zeros((1,), jnp.int32), rest])


@jax.jit
def _sa_select(q, pp):
    d2 = jnp.sum((q[:, None, :] - pp[None, :, :]) ** 2, -1)
    d2 = jnp.where(d2 <= RADIUS * RADIUS, d2, 1e30)
    neg, nbr = lax.top_k(-d2, K)
    valid = neg > -1e29
    return jnp.where(valid, nbr, nbr[:, :1]), valid


@jax.jit
def _fp_select(ps, pp):
    d2 = jnp.sum((ps[:, None, :] - pp[None, :, :]) ** 2, -1)
    neg, idx = lax.top_k(-d2, KNN)
    w = 1.0 / jnp.maximum(-neg, 1e-16)
    return idx, w / jnp.sum(w, axis=1, keepdims=True)


def _morton(p):
    q = ((p - p.min(0)) / (p.max(0) - p.min(0) + 1e-9) * 1023).astype(np.int64)

    def spread(x):
        x = (x | (x << 16)) & 0x030000FF
        x = (x | (x << 8)) & 0x0300F00F
        x = (x | (x << 4)) & 0x030C30C3
        x = (x | (x << 2)) & 0x09249249
        return x

    return spread(q[:, 0]) | (spread(q[:, 1]) << 1) | (spread(q[:, 2]) << 2)


def host_indices(pos):
    """Selection chain EXACTLY as the reference (FPS on reference-ordered
    points per level), then morton relabeling for the device tables.

    perm[l]: morton position -> reference position at level l.
    pos[l]: level-l positions in morton order.
    sa_wins[l]: per center (in level-(l+1) MORTON order) window stripe codes
    into the level-l morton table (2*j = pair (j, j+1), 2*j+1 = self (j, j)).
    fp_idx/fp_w: fine rows in fine-level morton order, ids into coarse morton.
    """
    posb = np.asarray(pos, np.float32).reshape(B, N0, 3)
    clouds = []
    with jax.default_device(_cpu):
        for b in range(B):
            # reference-order chain
            cur = posb[b]
            ref_pos = [cur]
            sa_nbr, sa_val = [], []
            for l in range(DEPTH):
                m = cur.shape[0] // 2
                idx = np.asarray(_fps_ref(jnp.asarray(cur), m))
                q = cur[idx]
                nbr, valid = _sa_select(jnp.asarray(q), jnp.asarray(cur))
                sa_nbr.append(np.asarray(nbr)); sa_val.append(np.asarray(valid))
                ref_pos.append(q)
                cur = q
            # morton perms per level
            perm = [np.argsort(_morton(p), kind="stable") for p in ref_pos]
            inv = []
            for p in perm:
                ip = np.empty_like(p); ip[p] = np.arange(p.size)
                inv.append(ip)
            cloud = {"perm": perm, "pos": [ref_pos[l][perm[l]] for l in range(DEPTH + 1)]}
            sa_wins = []
            for l in range(DEPTH):
                m = LEVELS[l + 1]
                wins = []
                for cm in range(m):
                    i = perm[l + 1][cm]
                    s = np.sort(inv[l][sa_nbr[l][i][sa_val[l][i]]])
                    runs = np.split(s, np.where(np.diff(s) != 1)[0] + 1)
                    w = []
                    for r in runs:
                        L = len(r)
                        a = int(r[0])
                        if L == 1:
                            w.append(2 * a + 1)
                        else:
                            starts = list(range(a, a + L, 2))
                            if L % 2 == 1:
                                starts[-1] = int(r[-1]) - 1
                            w.extend(2 * s0 for s0 in starts)
                    wins.append(np.asarray(w, np.int64))
                sa_wins.append(wins)
            fp_idx, fp_w = [], []
            for st in range(DEPTH):
                fine_l, coarse_l = DEPTH - 1 - st, DEPTH - st
                fine = jnp.asarray(cloud["pos"][fine_l])      # morton order
                coarse_ref = jnp.asarray(ref_pos[coarse_l])
                idx, wn = _fp_select(fine, coarse_ref)
                fp_idx.append(inv[coarse_l][np.asarray(idx)])  # -> coarse morton ids
                fp_w.append(np.asarray(wn))
            cloud.update(sa_wins=sa_wins, fp_idx=fp_idx, fp_w=fp_w)
            clouds.append(cloud)
    return clouds


def fold_weights(I):
    """Fold every BN affine and bias forward. Returns numpy fp32 dict."""
    s = BNS
    W = {"lin_w": np.asarray(I["lin_in_w"], np.float32),
         "lin_b": np.asarray(I["lin_in_b"], np.float32)}
    a_prev, b_prev = np.ones(NH, np.float32), np.zeros(NH, np.float32)
    pend = {0: (a_prev, b_prev)}
    for l in range(DEPTH):
        w1 = np.asarray(I["sa_w1"][l], np.float32)
        b1 = np.asarray(I["sa_b1"][l], np.float32)
        a1, be1 = np.asarray(I["sa_g1"][l], np.float32) * s, np.asarray(I["sa_be1"][l], np.float32)
        w2 = np.asarray(I["sa_w2"][l], np.float32)
        b2 = np.asarray(I["sa_b2"][l], np.float32)
        a2, be2 = np.asarray(I["sa_g2"][l], np.float32) * s, np.asarray(I["sa_be2"][l], np.float32)
        ap, bp = pend[l]
        w1f = w1.copy()
        w1f[:NH, :] = ap[:, None] * w1[:NH, :]
        b1f = b1 + bp @ w1[:NH, :]
        W[f"sa{l}"] = dict(w1_feat=w1f[:NH, :], w1_pos=w1f[NH:, :], b1=b1f,
                           w2=a1[:, None] * w2, b2=b2 + be1 @ w2)
        pend[l + 1] = (a2, be2)
    cur_coarse = pend[DEPTH]
    for st in range(DEPTH):
        mi = DEPTH - 1 - st
        fine_lvl = DEPTH - 1 - st
        w1 = np.asarray(I["fp_w1"][mi], np.float32)
        b1 = np.asarray(I["fp_b1"][mi], np.float32)
        a1, be1 = np.asarray(I["fp_g1"][mi], np.float32) * s, np.asarray(I["fp_be1"][mi], np.float32)
        w2 = np.asarray(I["fp_w2"][mi], np.float32)
        b2 = np.asarray(I["fp_b2"][mi], np.float32)
        a2, be2 = np.asarray(I["fp_g2"][mi], np.float32) * s, np.asarray(I["fp_be2"][mi], np.float32)
        ac, bc = cur_coarse
        as_, bs = pend[fine_lvl]
        W[f"fp{st}"] = dict(
            w1y=ac[:, None] * w1[:NH, :], w1s=as_[:, None] * w1[NH:, :],
            b1=b1 + bc @ w1[:NH, :] + bs @ w1[NH:, :],
            w2=a1[:, None] * w2, b2=b2 + be1 @ w2)
        cur_coarse = (a2, be2)
    w1 = np.asarray(I["lo_w1"], np.float32)
    b1 = np.asarray(I["lo_b1"], np.float32)
    a1, be1 = np.asarray(I["lo_g1"], np.float32) * s, np.asarray(I["lo_be1"], np.float32)
    w2 = np.asarray(I["lo_w2"], np.float32)
    b2 = np.asarray(I["lo_b2"], np.float32)
    af, bf = cur_coarse
    W["final"] = dict(w1=af[:, None] * w1, b1=b1 + bf @ w1,
                      w2=a1[:, None] * w2, b2=b2 + be1 @ w2,
                      a2=np.asarray(I["lo_g2"], np.float32) * s,
                      be2=np.asarray(I["lo_be2"], np.float32))
    return W


def wrap_idx(flat_idx):
    """int idx list -> [128, ceil(n/16)] int16 wrapped + replicated per Q7 core."""
    n = len(flat_idx)
    cols = (n + 15) // 16
    w = np.zeros((128, cols), np.int16)
    a = np.full(cols * 16, 0, np.int64)
    a[:n] = flat_idx
    w[:16, :] = a.reshape(cols, 16).T.astype(np.int16)
    for g in range(1, 8):
        w[g * 16:(g + 1) * 16] = w[:16]
    return w


def make_plan(clouds):
    """Pair-window chunk plan: per level, chunks of CC centers with Kp =
    envelope of window counts (over clouds), rounded up to a multiple of 2."""
    chunk_tables = []
    for l in range(DEPTH):
        m = LEVELS[l + 1]
        env = np.zeros(m, np.int64)
        for cl in clouds:
            env = np.maximum(env, [len(w) for w in cl["sa_wins"][l]])
        kps = []
        for i in range(0, m, CC):
            kp = int(env[i:i + CC].max())
            kps.append(max(4, ((kp + 3) // 4) * 4))
        # gather groups span 2 sub-chunks; Sg = CC*(kp_i + kp_{i+1}) must be
        # a multiple of 128 -> pair sums must be multiples of 8
        for i in range(0, len(kps) - 1, 2):
            if (kps[i] + kps[i + 1]) % 8:
                kps[i + 1] += 4
        tab = []
        off = 0
        for kp in kps:
            tab.append((CC, kp, off))
            off += CC * kp
        chunk_tables.append(tab)
    return chunk_tables


# ---------------------------------------------------------------- bass build

def build_kernel(chunk_tables):
    nc = bacc.Bacc(None, target_bir_lowering=False, debug=False)

    n_sa_idx = [sum(c * k for c, k, _ in chunk_tables[l]) for l in range(DEPTH)]
    nf_s = [LEVELS[DEPTH - 1 - s] for s in range(DEPTH)]  # FP fine sizes

    # ---------------- dram tensors
    xT = nc.dram_tensor("xT", [NIN, N0], BF16, kind="ExternalInput")
    posT = [nc.dram_tensor(f"posT{l}", [3, LEVELS[l]], BF16, kind="ExternalInput")
            for l in range(DEPTH + 1)]
    sa_idx = [nc.dram_tensor(f"sa_idx{l}", [128, (n_sa_idx[l] + 15) // 16], I16,
                             kind="ExternalInput")
              for l in range(DEPTH)]
    fp_idx = [nc.dram_tensor(f"fp_idx{s}", [128, nf_s[s] * KNN // 16], I16, kind="ExternalInput")
              for s in range(DEPTH)]
    fp_w = [nc.dram_tensor(f"fp_w{s}", [1, nf_s[s] * KNN], BF16, kind="ExternalInput")
            for s in range(DEPTH)]

    def wt(name, shape, dt=BF16):
        return nc.dram_tensor(name, shape, dt, kind="ExternalInput")

    lin_w = wt("lin_w", [NIN, NH])
    lin_b = wt("lin_b", [NH, 1], F32)
    saw = []
    for l in range(DEPTH):
        saw.append(dict(
            w1f=wt(f"sa{l}_w1f", [NH, D]),
            w1p=wt(f"sa{l}_w1p", [3, D]),
            w1pn6=wt(f"sa{l}_w1pn6", [3, NH + 6]),
            b1A=wt(f"sa{l}_b1A", [NH, 1], F32),
            b1B6=wt(f"sa{l}_b1B6", [6, 1], F32),
            w2A=wt(f"sa{l}_w2A", [NH, NH]),
            w2B=wt(f"sa{l}_w2B", [6, 2 * NH]),
            b2=wt(f"sa{l}_b2", [NH, 1], F32)))
    fpw = []
    for s in range(DEPTH):
        fpw.append(dict(
            w1y=wt(f"fp{s}_w1y", [NH, 2 * NH]),
            w1s=wt(f"fp{s}_w1s", [NH, 2 * NH]),
            b1a=wt(f"fp{s}_b1a", [NH, 1], F32),
            b1b=wt(f"fp{s}_b1b", [NH, 1], F32),
            w2a=wt(f"fp{s}_w2a", [NH, NH]),
            w2b=wt(f"fp{s}_w2b", [NH, NH]),
            b2=wt(f"fp{s}_b2", [NH, 1], F32)))
    fin = dict(w1=wt("fin_w1", [NH, NH]), b1=wt("fin_b1", [NH, 1], F32),
               w2=wt("fin_w2", [NH, NOUT]), b2=wt("fin_b2", [NOUT, 1], F32),
               a2=wt("fin_a2", [NOUT, 1], F32), be2=wt("fin_be2", [NOUT, 1], F32))

    outT = nc.dram_tensor("outT", [NOUT, N0], F32, kind="ExternalOutput")

    with tile.TileContext(nc) as tc:
        with tc.tile_pool(name="const", bufs=1) as constp, \
             tc.tile_pool(name="feat", bufs=1) as featp, \
             tc.tile_pool(name="ztab", bufs=1) as ztabp, \
             tc.tile_pool(name="wpool", bufs=1) as wp, \
             tc.tile_pool(name="gath", bufs=4) as gathp, \
             tc.tile_pool(name="gfp", bufs=2) as gfpp, \
             tc.tile_pool(name="h1", bufs=2) as h1p, \
             tc.tile_pool(name="idx", bufs=4) as idxp, \
             tc.tile_pool(name="small", bufs=2) as smallp, \
             tc.tile_pool(name="ps", bufs=2, space="PSUM") as ps, \
             tc.tile_pool(name="psu", bufs=4, space="PSUM") as psu:

            # ---- load inputs/weights to SBUF
            xT_t = constp.tile([NIN, N0], BF16)
            nc.sync.dma_start(xT_t[:], xT[:])
            posT3 = constp.tile([3, LEVELS[DEPTH]], BF16, tag="posT3")
            nc.sync.dma_start(posT3[:], posT[DEPTH][:])
            posx_t = []
            for l in range(DEPTH):
                t = constp.tile([3, LEVELS[0] + 8], BF16, tag="posx")
                nc.vector.memset(t[:, LEVELS[l]:LEVELS[l] + 8], 0.0)
                nc.sync.dma_start(t[:, :LEVELS[l]], posT[l][:])
                posx_t.append(t)

            def load_w(dram, shape, dt=BF16, tag=None):
                t = wp.tile(shape, dt, tag=tag or dram.name)
                nc.sync.dma_start(t[:], dram[:])
                return t

            lin_w_t = load_w(lin_w, [NIN, NH])
            lin_b_t = load_w(lin_b, [NH, 1], F32)
            saw_t = [{k: load_w(v, list(v.shape), v.dtype) for k, v in saw[l].items()}
                     for l in range(DEPTH)]
            fpw_t = [{k: load_w(v, list(v.shape), v.dtype) for k, v in fpw[s].items()}
                     for s in range(DEPTH)]
            fin_t = {k: load_w(v, list(v.shape), v.dtype) for k, v in fin.items()}

            ident = constp.tile([128, 128], BF16)
            from concourse.masks import make_identity
            make_identity(nc, ident[:])
            ones = constp.tile([1, 128], BF16, tag="ones")
            nc.vector.memset(ones[:], 1.0)

            # ---- lin_in: xb = relu(lin_w.T @ xT + lin_b) -> [128, 4097] bf16
            feats = []
            xb = featp.tile([NH, N0 + 1], BF16, tag="feat0")
            nc.vector.memset(xb[:, N0:], 0.0)
            for j in range(0, N0, 512):
                pt = psu.tile([NH, 512], F32, tag="mm")
                nc.tensor.matmul(pt[:], lin_w_t[:], xT_t[:, j:j + 512], start=True, stop=True)
                nc.scalar.activation(xb[:, j:j + 512], pt[:],
                                     mybir.ActivationFunctionType.Relu, bias=lin_b_t[:])
            feats.append(xb)

            # ---- SA levels
            for l in range(DEPTH):
                n, m = LEVELS[l], LEVELS[l + 1]
                w = saw_t[l]
                fprev, pprev = feats[l], posx_t[l]

                # shifted-by-one copies (aligned for PE moving operands)
                fshift = featp.tile([NH, LEVELS[0]], BF16, tag="fshift")
                pshift = featp.tile([3, LEVELS[0]], BF16, tag="pshift")
                nc.vector.tensor_copy(fshift[:, :n], fprev[:, 1:n + 1])
                nc.vector.tensor_copy(pshift[:, :n], pprev[:, 1:n + 1])
                # pair-stripe z table: 2n stripes x 384 elems.
                # stripe 2j (pair) lives at partition (2j)%128... NO: stripe id s
                # -> partition s%128, rank s//128. We instead INTERLEAVE ids so
                # content stays partition-aligned: stripe for pair-of-j has id
                # chosen by the HOST as j + n*(0) with ranks; see layout below.
                # Layout: ranks 0..n/128-1 hold PAIR stripes (rows j, j+1) with
                # id = j;   ranks n/128 .. 2n/128-1 hold SELF stripes (j, j)
                # with id = n + j.  Partition of id j is j%128 = row j's psum
                # partition for every write.
                ztab = ztabp.tile([128, (2 * LEVELS[0] // 128) * STRIPE], BF16, tag="ztab")
                for pc in range(n // 128):
                    # z rows for points j = pc*128 .. pc*128+127
                    pz = psu.tile([128, 512], F32, tag="mm")
                    nc.tensor.matmul(pz[:, :D], fprev[:, pc * 128:(pc + 1) * 128],
                                     w["w1f"][:], start=True, stop=False)
                    nc.tensor.matmul(pz[:, :D], pprev[:, pc * 128:(pc + 1) * 128],
                                     w["w1p"][:], start=False, stop=True)
                    # shifted: rows j+1 (reads one column beyond at the edge;
                    # feat/pos tables carry a zero pad column)
                    pzs = psu.tile([128, 512], F32, tag="mm")
                    nc.tensor.matmul(pzs[:, :D], fshift[:, pc * 128:(pc + 1) * 128],
                                     w["w1f"][:], start=True, stop=False)
                    nc.tensor.matmul(pzs[:, :D], pshift[:, pc * 128:(pc + 1) * 128],
                                     w["w1p"][:], start=False, stop=True)
                    o = pc * STRIPE
                    so = (n // 128) * STRIPE + pc * STRIPE
                    A = mybir.ActivationFunctionType.Identity
                    # pair stripe: [zA_j | zA_j1 | zB_j zB_j1 pad]
                    nc.scalar.activation(ztab[:, o:o + 128], pz[:, :128], A)
                    nc.scalar.activation(ztab[:, o + 128:o + 256], pzs[:, :128], A)
                    nc.scalar.activation(ztab[:, o + 256:o + 259], pz[:, 128:131], A)
                    nc.scalar.activation(ztab[:, o + 259:o + 262], pzs[:, 128:131], A)
                    # self stripe: [zA_j | zA_j | zB_j zB_j pad]
                    nc.scalar.activation(ztab[:, so:so + 128], pz[:, :128], A)
                    nc.scalar.activation(ztab[:, so + 128:so + 256], pz[:, :128], A)
                    nc.scalar.activation(ztab[:, so + 256:so + 259], pz[:, 128:131], A)
                    nc.scalar.activation(ztab[:, so + 259:so + 262], pz[:, 128:131], A)

                # c tables: cA [128, m], cB6 [6, m] (tail bias doubled)
                q_t = posx_t[l + 1] if l < 2 else posT3
                cA = featp.tile([NH, LEVELS[1]], BF16, tag="cA")
                cB6 = featp.tile([6, LEVELS[1]], BF16, tag="cB")
                for j in range(0, m, 512):
                    e = min(512, m - j)
                    pcA = psu.tile([NH, 512], F32, tag="mm")
                    nc.tensor.matmul(pcA[:, :e], w["w1pn6"][:, :NH], q_t[:, j:j + e],
                                     start=True, stop=True)
                    nc.scalar.activation(cA[:, j:j + e], pcA[:, :e],
                                         mybir.ActivationFunctionType.Identity,
                                         bias=w["b1A"][:])
                    pcB = ps.tile([6, 512], F32, tag="sm")
                    nc.tensor.matmul(pcB[:, :e], w["w1pn6"][:, NH:], q_t[:, j:j + e],
                                     start=True, stop=True)
                    nc.scalar.activation(cB6[:, j:j + e], pcB[:, :e],
                                         mybir.ActivationFunctionType.Identity,
                                         bias=w["b1B6"][:])

                # main: per chunk gather pairs + h1(e/o/t) + layer2 + maxpool
                hmax = featp.tile([NH, LEVELS[1]], BF16, tag="hmax")
                cpos = 0
                tab = chunk_tables[l]
                for gi in range(0, len(tab), 2):
                  grp = tab[gi:gi + 2]
                  Sg = sum(Cg * Kg for Cg, Kg, _ in grp)
                  gioff = grp[0][2]
                  gcols16 = (Sg + 15) // 16
                  it = idxp.tile([128, gcols16], I16, tag="idx")
                  nc.sync.dma_start(it[:], sa_idx[l][:, gioff // 16:gioff // 16 + gcols16])
                  g2 = gathp.tile([128, 3, Sg], BF16, tag="gsa")
                  nc.gpsimd.dma_gather(
                      out_ap=g2[:], in_ap=ztab[:], idxs_ap=it[:],
                      num_idxs=Sg, num_idxs_reg=Sg, elem_size=STRIPE,
                      transpose=True, single_packet=False,
                      sbuf_tokens_per_rank=128, sbuf_free_dim_per_rank=STRIPE * 2)
                  sbase = 0
                  for (C, Kp, ioff) in grp:
                    S = C * Kp
                    sb = sbase
                    sbase += S
                    h1e = h1p.tile([NH, S], BF16, tag="h1e")
                    h1o = h1p.tile([NH, S], BF16, tag="h1o")
                    h1t = h1p.tile([8, S], BF16, tag="h1t")
                    for half, ht in ((0, h1e), (1, h1o)):
                        nc.vector.tensor_tensor(
                            out=ht[:].rearrange("p (c k) -> p c k", k=Kp),
                            in0=g2[:, half, sb:sb + S].rearrange("p (c k) -> p c k", k=Kp),
                            in1=cA[:, cpos:cpos + C][:, :, None].broadcast_to([NH, C, Kp]),
                            op=mybir.AluOpType.add)
                        nc.scalar.activation(ht[:], ht[:], mybir.ActivationFunctionType.Relu)
                    nc.vector.tensor_tensor(
                        out=h1t[:6].rearrange("p (c k) -> p c k", k=Kp),
                        in0=g2[:6, 2, sb:sb + S].rearrange("p (c k) -> p c k", k=Kp),
                        in1=cB6[:, cpos:cpos + C][:, :, None].broadcast_to([6, C, Kp]),
                        op=mybir.AluOpType.add)
                    nc.scalar.activation(h1t[:6], h1t[:6], mybir.ActivationFunctionType.Relu)
                    # layer 2 + maxpool per psum-sized slice
                    Npc = (512 // Kp) * Kp
                    for s0 in range(0, S, Npc):
                        e = min(Npc, S - s0)
                        cc = e // Kp
                        col = cpos + s0 // Kp
                        pe = psu.tile([NH, 512], F32, tag="mm")
                        nc.tensor.matmul(pe[:, :e], w["w2A"][:], h1e[:, s0:s0 + e],
                                         start=True, stop=False)
                        nc.tensor.matmul(pe[:, :e], w["w2B"][:, :NH], h1t[:6, s0:s0 + e],
                                         start=False, stop=True)
                        po = psu.tile([NH, 512], F32, tag="mm")
                        nc.tensor.matmul(po[:, :e], w["w2A"][:], h1o[:, s0:s0 + e],
                                         start=True, stop=False)
                        nc.tensor.matmul(po[:, :e], w["w2B"][:, NH:], h1t[:6, s0:s0 + e],
                                         start=False, stop=True)
                        rme = smallp.tile([NH, 128], BF16, tag="rme")
                        nc.vector.tensor_reduce(
                            out=rme[:, :cc],
                            in_=pe[:, :e].rearrange("p (c k) -> p c k", k=Kp),
                            axis=mybir.AxisListType.X, op=mybir.AluOpType.max)
                        rmo = smallp.tile([NH, 128], BF16, tag="rmo")
                        nc.vector.tensor_reduce(
                            out=rmo[:, :cc],
                            in_=po[:, :e].rearrange("p (c k) -> p c k", k=Kp),
                            axis=mybir.AxisListType.X, op=mybir.AluOpType.max)
                        nc.vector.tensor_tensor(out=hmax[:, col:col + cc],
                                                in0=rme[:, :cc], in1=rmo[:, :cc],
                                                op=mybir.AluOpType.max)
                    cpos += C
                # x~_l = relu(hmax + b2); one pad col for the next shifted pass
                xl = featp.tile([NH, m + 1], BF16, tag=f"feat{l + 1}")
                nc.vector.memset(xl[:, m:], 0.0)
                for j in range(0, m, 2048):
                    e = min(2048, m - j)
                    nc.scalar.activation(xl[:, j:j + e], hmax[:, j:j + e],
                                         mybir.ActivationFunctionType.Relu,
                                         bias=w["b2"][:])
                feats.append(xl)

            # ---- FP stages (chunked: 512 fine points / 1536 idxs at a time)
            xf = feats[DEPTH]
            for s in range(DEPTH):
                nf = nf_s[s]
                ncoarse = LEVELS[DEPTH - s]
                w = fpw_t[s]
                ftab = ztabp.tile([128, (LEVELS[1] // 128) * 128], BF16, tag="ftab")
                for r in range(ncoarse // 128):
                    ptr = ps.tile([128, 128], BF16, tag="tt")
                    nc.tensor.transpose(ptr[:], xf[:, r * 128:(r + 1) * 128], ident[:])
                    nc.vector.tensor_copy(ftab[:, r * 128:(r + 1) * 128], ptr[:])
                xfn = featp.tile([NH, nf + 1], BF16, tag=f"fpout{s}")
                nc.vector.memset(xfn[:, nf:], 0.0)
                skip = feats[DEPTH - 1 - s]
                for j in range(0, nf, 512):
                    jj = j * KNN
                    it = idxp.tile([128, 1536 // 16], I16, tag="idxfp")
                    nc.sync.dma_start(it[:], fp_idx[s][:, jj // 16:(jj + 1536) // 16])
                    g = gfpp.tile([128, 1, 1536], BF16, tag="gfp")
                    nc.gpsimd.dma_gather(
                        out_ap=g[:], in_ap=ftab[:], idxs_ap=it[:],
                        num_idxs=1536, num_idxs_reg=1536, elem_size=128,
                        transpose=True, single_packet=False,
                        sbuf_tokens_per_rank=128, sbuf_free_dim_per_rank=256)
                    wrow = wp.tile([1, 1536], BF16, tag="wrow")
                    nc.sync.dma_start(wrow[:], fp_w[s][:, jj:jj + 1536])
                    wgc = h1p.tile([NH, 1536], BF16, tag="wgc")
                    for j2 in range(0, 1536, 512):
                        pw = psu.tile([128, 512], F32, tag="mm")
                        nc.tensor.matmul(pw[:], ones[:], wrow[:, j2:j2 + 512],
                                         start=True, stop=True)
                        nc.vector.tensor_tensor(out=wgc[:, j2:j2 + 512],
                                                in0=g[:, 0, j2:j2 + 512],
                                                in1=pw[:], op=mybir.AluOpType.mult)
                    w3 = wgc[:].rearrange("p (n k) -> p n k", k=KNN)
                    y = h1p.tile([NH, 512], BF16, tag="y")
                    nc.vector.tensor_tensor(out=y[:], in0=w3[:, :, 0], in1=w3[:, :, 1],
                                            op=mybir.AluOpType.add)
                    nc.vector.tensor_tensor(out=y[:], in0=y[:], in1=w3[:, :, 2],
                                            op=mybir.AluOpType.add)
                    h1a = h1p.tile([NH, 512], BF16, tag="fph1a")
                    h1b = h1p.tile([NH, 512], BF16, tag="fph1b")
                    pa = psu.tile([NH, 512], F32, tag="mm")
                    nc.tensor.matmul(pa[:], w["w1y"][:, :NH], y[:], start=True, stop=False)
                    nc.tensor.matmul(pa[:], w["w1s"][:, :NH], skip[:, j:j + 512],
                                     start=False, stop=True)
                    nc.scalar.activation(h1a[:], pa[:],
                                         mybir.ActivationFunctionType.Relu, bias=w["b1a"][:])
                    pb = psu.tile([NH, 512], F32, tag="mm")
                    nc.tensor.matmul(pb[:], w["w1y"][:, NH:], y[:], start=True, stop=False)
                    nc.tensor.matmul(pb[:], w["w1s"][:, NH:], skip[:, j:j + 512],
                                     start=False, stop=True)
                    nc.scalar.activation(h1b[:], pb[:],
                                         mybir.ActivationFunctionType.Relu, bias=w["b1b"][:])
                    pc2 = psu.tile([NH, 512], F32, tag="mm")
                    nc.tensor.matmul(pc2[:], w["w2a"][:], h1a[:], start=True, stop=False)
                    nc.tensor.matmul(pc2[:], w["w2b"][:], h1b[:], start=False, stop=True)
                    nc.scalar.activation(xfn[:, j:j + 512], pc2[:],
                                         mybir.ActivationFunctionType.Relu, bias=w["b2"][:])
                xf = xfn

            # ---- final head
            for j in range(0, N0, 256):
                ph = psu.tile([NH, 512], F32, tag="mm")
                nc.tensor.matmul(ph[:, :256], fin_t["w1"][:], xf[:, j:j + 256], start=True, stop=True)
                hh = h1p.tile([NH, 256], BF16, tag="finh")
                nc.scalar.activation(hh[:], ph[:, :256], mybir.ActivationFunctionType.Relu,
                                     bias=fin_t["b1"][:])
                po = ps.tile([NOUT, 256], F32, tag="sm")
                nc.tensor.matmul(po[:], fin_t["w2"][:], hh[:], start=True, stop=True)
                oo = h1p.tile([NOUT, 256], F32, tag="fino")
                nc.scalar.activation(oo[:], po[:], mybir.ActivationFunctionType.Relu,
                                     bias=fin_t["b2"][:])
                nc.vector.tensor_scalar(out=oo[:], in0=oo[:],
                                        scalar1=fin_t["a2"][:], scalar2=fin_t["be2"][:],
                                        op0=mybir.AluOpType.mult, op1=mybir.AluOpType.add)
                nc.sync.dma_start(outT[:, j:j + 256], oo[:])

    nc.compile()
    return nc


# ---------------------------------------------------------------- packaging

def make_core_inputs(xc, cloud, W, chunk_tables):
    """Per-core inputs for one cloud.  xc: [N0, NIN] in ORIGINAL row order.
    All level orders are the cloud's morton orders; stripe id of pair (j,j+1)
    at level l is j, self stripe (j,j) is n + j."""
    d = {}
    perm0 = cloud["perm"][0]
    d["xT"] = np.ascontiguousarray(xc[perm0].T).astype(bf16)
    for l in range(DEPTH + 1):
        d[f"posT{l}"] = np.ascontiguousarray(cloud["pos"][l].T).astype(bf16)
    for l in range(DEPTH):
        n = LEVELS[l]
        wins = cloud["sa_wins"][l]
        flat = []
        cpos = 0
        for (C, Kp, _off) in chunk_tables[l]:
            for ci in range(cpos, cpos + C):
                w = wins[ci]
                # stripe id: even ids are pairs (id 2j -> stripe j at rank j//128),
                # odd are self (2j+1 -> stripe n + j).  Convert:
                sid = np.where(w % 2 == 0, w // 2, n + (w - 1) // 2)
                pad = np.full(Kp - len(sid), sid[0] if len(sid) else n, np.int64)
                flat.append(np.concatenate([sid, pad]))
            cpos += C
        flat = np.concatenate(flat)
        d[f"sa_idx{l}"] = wrap_idx(flat)
    for s in range(DEPTH):
        idxr = cloud["fp_idx"][s]            # already in morton orders
        d[f"fp_idx{s}"] = wrap_idx(idxr.reshape(-1))
        d[f"fp_w{s}"] = cloud["fp_w"][s].reshape(1, -1).astype(bf16)
    d["lin_w"] = W["lin_w"].astype(bf16)
    d["lin_b"] = W["lin_b"].reshape(NH, 1).astype(np.float32)
    for l in range(DEPTH):
        Wl = W[f"sa{l}"]
        d[f"sa{l}_w1f"] = Wl["w1_feat"].astype(bf16)
        d[f"sa{l}_w1p"] = Wl["w1_pos"].astype(bf16)
        wpn = -Wl["w1_pos"]                   # [3, 131]
        wpn6 = np.concatenate([wpn[:, :NH], wpn[:, NH:], wpn[:, NH:]], axis=1)
        d[f"sa{l}_w1pn6"] = wpn6.astype(bf16)  # [3, 134]
        d[f"sa{l}_b1A"] = Wl["b1"][:NH].reshape(NH, 1).astype(np.float32)
        d[f"sa{l}_b1B6"] = np.concatenate([Wl["b1"][NH:], Wl["b1"][NH:]]).reshape(6, 1).astype(np.float32)
        d[f"sa{l}_w2A"] = Wl["w2"][:NH, :].astype(bf16)
        w2b6 = np.zeros((6, 2 * NH), np.float32)
        w2b6[0:3, :NH] = Wl["w2"][NH:, :]      # even bank: rows 0..2
        w2b6[3:6, NH:] = Wl["w2"][NH:, :]      # odd bank: rows 3..5
        d[f"sa{l}_w2B"] = w2b6.astype(bf16)
        d[f"sa{l}_b2"] = Wl["b2"].reshape(NH, 1).astype(np.float32)
    for s in range(DEPTH):
        Ws = W[f"fp{s}"]
        d[f"fp{s}_w1y"] = Ws["w1y"].astype(bf16)
        d[f"fp{s}_w1s"] = Ws["w1s"].astype(bf16)
        d[f"fp{s}_b1a"] = Ws["b1"][:NH].reshape(NH, 1).astype(np.float32)
        d[f"fp{s}_b1b"] = Ws["b1"][NH:].reshape(NH, 1).astype(np.float32)
        d[f"fp{s}_w2a"] = Ws["w2"][:NH, :].astype(bf16)
        d[f"fp{s}_w2b"] = Ws["w2"][NH:, :].astype(bf16)
        d[f"fp{s}_b2"] = Ws["b2"].reshape(NH, 1).astype(np.float32)
    Wf = W["final"]
    d["fin_w1"] = Wf["w1"].astype(bf16)
    d["fin_b1"] = Wf["b1"].reshape(NH, 1).astype(np.float32)
    d["fin_w2"] = Wf["w2"].astype(bf16)
    d["fin_b2"] = Wf["b2"].reshape(NOUT, 1).astype(np.float32)
    d["fin_a2"] = Wf["a2"].reshape(NOUT, 1).astype(np.float32)
    d["fin_be2"] = Wf["be2"].reshape(NOUT, 1).astype(np.float32)
    return d


_KERNEL_CACHE = {}


def kernel(trace=False, **inputs):
    clouds = host_indices(np.asarray(inputs["pos"], np.float32))
    W = fold_weights(inputs)
    chunk_tables = make_plan(clouds)
    key = tuple(tuple(t) for tab in chunk_tables for t in tab)
    if key not in _KERNEL_CACHE:
        _KERNEL_CACHE.clear()
        _KERNEL_CACHE[key] = build_kernel(chunk_tables)
    nc = _KERNEL_CACHE[key]
    xr = np.asarray(inputs["x"], np.float32).reshape(B, N0, NIN)
    in_maps = []
    for core in range(8):
        b = core // 2
        in_maps.append(make_core_inputs(xr[b], clouds[b], W, chunk_tables))
    res = run_bass_kernel_spmd(nc, in_maps, core_ids=list(range(8)), trace=trace)
    outs = []
    for b in range(B):
        ot = np.ascontiguousarray(res.results[2 * b]["outT"].T)   # [N0, 2] morton
        o = np.empty_like(ot)
        o[clouds[b]["perm"][0]] = ot                               # undo morton
        outs.append(o)
    out = np.concatenate(outs, 0).astype(np.float32)
    if trace:
        kernel.last_exec_time_ns = res.exec_time_ns
        kernel.last_trace = res.instructions_and_trace
    return out


kernel.last_exec_time_ns = None
kernel.last_trace = None



# revision 28
# speedup vs baseline: 1.4443x; 1.4443x over previous
"""PointNet++ (SA x3 + FP x3 + head) on 8 Trainium2 NeuronCores via Bass/Tile.

Strategy (v2: morton pair-window gathers)
-----------------------------------------
- Host: FPS / radius-kNN / kNN-interp selection (pos-derived only), morton
  sort of every level's points, pair-window covers of each center's exact
  neighbor set, BN/bias folding, index wrapping.
- Device (per core, channel-major): as before, but the SA neighbor gather
  uses a "pair stripe" z table: stripe 2j = z rows (j, j+1) packed compact
  (768B: zA_j | zA_{j+1} | zB_j zB_{j+1} pad), stripe 2j+1 = (j, j).
  Morton ordering makes neighbor sets contiguous runs, so ~2 slots ride on
  every gather index (the SWDGE gather costs ~7.7ns/index regardless of
  element bytes up to ~1KB). Odd runs overlap-duplicate; isolated rows use
  the self stripe; duplicates never change a max.
- Sharding: cores 2b and 2b+1 both compute cloud b; host takes even cores.
"""

import numpy as np
import ml_dtypes

import jax
import jax.numpy as jnp
from jax import lax

import concourse.bass as bass
import concourse.mybir as mybir
from concourse import bacc
import concourse.tile as tile
from concourse.bass_utils import run_bass_kernel_spmd

# ---------------------------------------------------------------- constants
B, N0, NIN, NH, NOUT = 4, 4096, 16, 128, 2
DEPTH, K, RADIUS, KNN, EPS = 3, 64, 2.0, 3, 1e-5
D = NH + 3        # 131, PointConv MLP width
BNS = np.float32(1.0 / np.sqrt(1.0 + EPS))
BF16 = mybir.dt.bfloat16
F32 = mybir.dt.float32
I16 = mybir.dt.int16
bf16 = ml_dtypes.bfloat16

LEVELS = [N0 // (2 ** i) for i in range(DEPTH + 1)]   # [4096, 2048, 1024, 512]
CC = 16           # centers per sub-chunk (2 sub-chunks per gather)
STRIPE = 384      # bf16 elems per pair stripe (768B)

_cpu = jax.devices("cpu")[0]

# ---------------------------------------------------------------- host prep

def _fps_ref(pts, m):
    n = pts.shape[0]

    def step(carry, _):
        dmin, last = carry
        d = jnp.sum((pts - pts[last]) ** 2, axis=-1)
        dmin = jnp.minimum(dmin, d)
        nxt = jnp.argmax(dmin).astype(jnp.int32)
        return (dmin, nxt), nxt

    _, rest = lax.scan(step, (jnp.full((n,), 1e30, pts.dtype), jnp.int32(0)),
                       None, length=m - 1)
    return jnp.concatenate([jnp.zeros((1,), jnp.int32), rest])


@jax.jit
def _sa_select(q, pp):
    d2 = jnp.sum((q[:, None, :] - pp[None, :, :]) ** 2, -1)
    d2 = jnp.where(d2 <= RADIUS * RADIUS, d2, 1e30)
    neg, nbr = lax.top_k(-d2, K)
    valid = neg > -1e29
    return jnp.where(valid, nbr, nbr[:, :1]), valid


@jax.jit
def _fp_select(ps, pp):
    d2 = jnp.sum((ps[:, None, :] - pp[None, :, :]) ** 2, -1)
    neg, idx = lax.top_k(-d2, KNN)
    w = 1.0 / jnp.maximum(-neg, 1e-16)
    return idx, w / jnp.sum(w, axis=1, keepdims=True)


def _morton(p):
    q = ((p - p.min(0)) / (p.max(0) - p.min(0) + 1e-9) * 1023).astype(np.int64)

    def spread(x):
        x = (x | (x << 16)) & 0x030000FF
        x = (x | (x << 8)) & 0x0300F00F
        x = (x | (x << 4)) & 0x030C30C3
        x = (x | (x << 2)) & 0x09249249
        return x

    return spread(q[:, 0]) | (spread(q[:, 1]) << 1) | (spread(q[:, 2]) << 2)


def host_indices(pos):
    """Selection chain EXACTLY as the reference (FPS on reference-ordered
    points per level), then morton relabeling for the device tables.

    perm[l]: morton position -> reference position at level l.
    pos[l]: level-l positions in morton order.
    sa_wins[l]: per center (in level-(l+1) MORTON order) window stripe codes
    into the level-l morton table (2*j = pair (j, j+1), 2*j+1 = self (j, j)).
    fp_idx/fp_w: fine rows in fine-level morton order, ids into coarse morton.
    """
    posb = np.asarray(pos, np.float32).reshape(B, N0, 3)
    clouds = []
    with jax.default_device(_cpu):
        for b in range(B):
            # reference-order chain
            cur = posb[b]
            ref_pos = [cur]
            sa_nbr, sa_val = [], []
            for l in range(DEPTH):
                m = cur.shape[0] // 2
                idx = np.asarray(_fps_ref(jnp.asarray(cur), m))
                q = cur[idx]
                nbr, valid = _sa_select(jnp.asarray(q), jnp.asarray(cur))
                sa_nbr.append(np.asarray(nbr)); sa_val.append(np.asarray(valid))
                ref_pos.append(q)
                cur = q
            # morton perms per level
            perm = [np.argsort(_morton(p), kind="stable") for p in ref_pos]
            inv = []
            for p in perm:
                ip = np.empty_like(p); ip[p] = np.arange(p.size)
                inv.append(ip)
            cloud = {"perm": perm, "pos": [ref_pos[l][perm[l]] for l in range(DEPTH + 1)]}
            sa_wins = []
            for l in range(DEPTH):
                m = LEVELS[l + 1]
                wins = []
                for cm in range(m):
                    i = perm[l + 1][cm]
                    s = np.sort(inv[l][sa_nbr[l][i][sa_val[l][i]]])
                    runs = np.split(s, np.where(np.diff(s) != 1)[0] + 1)
                    w = []
                    for r in runs:
                        L = len(r)
                        a = int(r[0])
                        if L == 1:
                            w.append(2 * a + 1)
                        else:
                            starts = list(range(a, a + L, 2))
                            if L % 2 == 1:
                                starts[-1] = int(r[-1]) - 1
                            w.extend(2 * s0 for s0 in starts)
                    wins.append(np.asarray(w, np.int64))
                sa_wins.append(wins)
            fp_idx, fp_w = [], []
            for st in range(DEPTH):
                fine_l, coarse_l = DEPTH - 1 - st, DEPTH - st
                fine = jnp.asarray(cloud["pos"][fine_l])      # morton order
                coarse_ref = jnp.asarray(ref_pos[coarse_l])
                idx, wn = _fp_select(fine, coarse_ref)
                fp_idx.append(inv[coarse_l][np.asarray(idx)])  # -> coarse morton ids
                fp_w.append(np.asarray(wn))
            cloud.update(sa_wins=sa_wins, fp_idx=fp_idx, fp_w=fp_w)
            clouds.append(cloud)
    return clouds


def fold_weights(I):
    """Fold every BN affine and bias forward. Returns numpy fp32 dict."""
    s = BNS
    W = {"lin_w": np.asarray(I["lin_in_w"], np.float32),
         "lin_b": np.asarray(I["lin_in_b"], np.float32)}
    a_prev, b_prev = np.ones(NH, np.float32), np.zeros(NH, np.float32)
    pend = {0: (a_prev, b_prev)}
    for l in range(DEPTH):
        w1 = np.asarray(I["sa_w1"][l], np.float32)
        b1 = np.asarray(I["sa_b1"][l], np.float32)
        a1, be1 = np.asarray(I["sa_g1"][l], np.float32) * s, np.asarray(I["sa_be1"][l], np.float32)
        w2 = np.asarray(I["sa_w2"][l], np.float32)
        b2 = np.asarray(I["sa_b2"][l], np.float32)
        a2, be2 = np.asarray(I["sa_g2"][l], np.float32) * s, np.asarray(I["sa_be2"][l], np.float32)
        ap, bp = pend[l]
        w1f = w1.copy()
        w1f[:NH, :] = ap[:, None] * w1[:NH, :]
        b1f = b1 + bp @ w1[:NH, :]
        W[f"sa{l}"] = dict(w1_feat=w1f[:NH, :], w1_pos=w1f[NH:, :], b1=b1f,
                           w2=a1[:, None] * w2, b2=b2 + be1 @ w2)
        pend[l + 1] = (a2, be2)
    cur_coarse = pend[DEPTH]
    for st in range(DEPTH):
        mi = DEPTH - 1 - st
        fine_lvl = DEPTH - 1 - st
        w1 = np.asarray(I["fp_w1"][mi], np.float32)
        b1 = np.asarray(I["fp_b1"][mi], np.float32)
        a1, be1 = np.asarray(I["fp_g1"][mi], np.float32) * s, np.asarray(I["fp_be1"][mi], np.float32)
        w2 = np.asarray(I["fp_w2"][mi], np.float32)
        b2 = np.asarray(I["fp_b2"][mi], np.float32)
        a2, be2 = np.asarray(I["fp_g2"][mi], np.float32) * s, np.asarray(I["fp_be2"][mi], np.float32)
        ac, bc = cur_coarse
        as_, bs = pend[fine_lvl]
        W[f"fp{st}"] = dict(
            w1y=ac[:, None] * w1[:NH, :], w1s=as_[:, None] * w1[NH:, :],
            b1=b1 + bc @ w1[:NH, :] + bs @ w1[NH:, :],
            w2=a1[:, None] * w2, b2=b2 + be1 @ w2)
        cur_coarse = (a2, be2)
    w1 = np.asarray(I["lo_w1"], np.float32)
    b1 = np.asarray(I["lo_b1"], np.float32)
    a1, be1 = np.asarray(I["lo_g1"], np.float32) * s, np.asarray(I["lo_be1"], np.float32)
    w2 = np.asarray(I["lo_w2"], np.float32)
    b2 = np.asarray(I["lo_b2"], np.float32)
    af, bf = cur_coarse
    W["final"] = dict(w1=af[:, None] * w1, b1=b1 + bf @ w1,
                      w2=a1[:, None] * w2, b2=b2 + be1 @ w2,
                      a2=np.asarray(I["lo_g2"], np.float32) * s,
                      be2=np.asarray(I["lo_be2"], np.float32))
    return W


def wrap_idx(flat_idx):
    """int idx list -> [128, ceil(n/16)] int16 wrapped + replicated per Q7 core."""
    n = len(flat_idx)
    cols = (n + 15) // 16
    w = np.zeros((128, cols), np.int16)
    a = np.full(cols * 16, 0, np.int64)
    a[:n] = flat_idx
    w[:16, :] = a.reshape(cols, 16).T.astype(np.int16)
    for g in range(1, 8):
        w[g * 16:(g + 1) * 16] = w[:16]
    return w


def make_plan(clouds):
    """Pair-window chunk plan: per level, chunks of CC centers with Kp =
    envelope of window counts (over clouds), rounded up to a multiple of 2."""
    chunk_tables = []
    for l in range(DEPTH):
        m = LEVELS[l + 1]
        env = np.zeros(m, np.int64)
        for cl in clouds:
            env = np.maximum(env, [len(w) for w in cl["sa_wins"][l]])
        # 2-way core split: chunk slot i covers centers [i*CC,(i+1)*CC) of the
        # core's half; Kp envelope over clouds AND the two halves.
        envh = np.maximum(env[:m // 2], env[m // 2:])
        kps = []
        for i in range(0, m // 2, CC):
            kp = int(envh[i:i + CC].max())
            kps.append(max(4, ((kp + 3) // 4) * 4))
        # gather groups span 2 sub-chunks; Sg = CC*(kp_i + kp_{i+1}) must be
        # a multiple of 128 -> pair sums must be multiples of 8
        for i in range(0, len(kps) - 1, 2):
            if (kps[i] + kps[i + 1]) % 8:
                kps[i + 1] += 4
        tab = []
        off = 0
        for kp in kps:
            tab.append((CC, kp, off))
            off += CC * kp
        chunk_tables.append(tab)
    return chunk_tables


CHF = 512  # fine points per FP interpolation chunk


def make_fp_plan(clouds):
    """FP dense-S plan: per stage, per CHF-fine chunk, the 128-aligned coarse
    morton window [base, base+W) (envelope over clouds) whose rows carry
    interpolation weights.  Returns per stage a list of (base, W, coloff)."""
    fp_tabs = []
    for s in range(DEPTH):
        nf = LEVELS[DEPTH - 1 - s]
        ncoarse = LEVELS[DEPTH - s]
        tab = []
        coloff = 0
        for j in range(0, nf, CHF):
            lo, hi = ncoarse, -1
            for cl in clouds:
                blk = cl["fp_idx"][s].reshape(nf, KNN)[j:j + CHF]
                lo = min(lo, int(blk.min()))
                hi = max(hi, int(blk.max()))
            base = (lo // 128) * 128
            W = ((hi - base) // 128 + 1) * 128
            tab.append((base, W, coloff))
            coloff += (W // 128) * CHF
        fp_tabs.append(tab)
    return fp_tabs


# ---------------------------------------------------------------- bass build

def build_kernel(chunk_tables, fp_tabs):
    nc = bacc.Bacc(None, target_bir_lowering=False, debug=False)

    n_sa_idx = [sum(c * k for c, k, _ in chunk_tables[l]) for l in range(DEPTH)]
    nf_s = [LEVELS[DEPTH - 1 - s] for s in range(DEPTH)]  # FP fine sizes

    # ---------------- dram tensors
    xT = nc.dram_tensor("xT", [NIN, N0], BF16, kind="ExternalInput")
    posT = [nc.dram_tensor(f"posT{l}", [3, LEVELS[l]], BF16, kind="ExternalInput")
            for l in range(DEPTH)]
    posQ = [nc.dram_tensor(f"posQ{l}", [3, LEVELS[l + 1] // 2], BF16,
                           kind="ExternalInput")
            for l in range(DEPTH)]
    sa_idx = [nc.dram_tensor(f"sa_idx{l}", [128, (n_sa_idx[l] + 15) // 16], I16,
                             kind="ExternalInput")
              for l in range(DEPTH)]
    n_fp_cols = [fp_tabs[s][-1][2] + (fp_tabs[s][-1][1] // 128) * CHF
                 for s in range(DEPTH)]
    fp_S = [nc.dram_tensor(f"fp_S{s}", [128, n_fp_cols[s]], BF16, kind="ExternalInput")
            for s in range(DEPTH)]

    def wt(name, shape, dt=BF16):
        return nc.dram_tensor(name, shape, dt, kind="ExternalInput")

    lin_w = wt("lin_w", [NIN, NH])
    lin_b = wt("lin_b", [NH, 1], F32)
    saw = []
    for l in range(DEPTH):
        saw.append(dict(
            w1f=wt(f"sa{l}_w1f", [NH, D]),
            w1p=wt(f"sa{l}_w1p", [3, D]),
            w1p134=wt(f"sa{l}_w1p134", [3, NH + 6]),
            b1An=wt(f"sa{l}_b1An", [NH, 1], F32),
            b1B6n=wt(f"sa{l}_b1B6n", [6, 1], F32),
            w2A=wt(f"sa{l}_w2A", [NH, NH]),
            w2B=wt(f"sa{l}_w2B", [6, 2 * NH]),
            w2t3=wt(f"sa{l}_w2t3", [3, NH]),
            negb2=wt(f"sa{l}_negb2", [NH, 1], F32)))
    fpw = []
    for s in range(DEPTH):
        fpw.append(dict(
            w1y=wt(f"fp{s}_w1y", [NH, 2 * NH]),
            w1s=wt(f"fp{s}_w1s", [NH, 2 * NH]),
            b1a=wt(f"fp{s}_b1a", [NH, 1], F32),
            b1b=wt(f"fp{s}_b1b", [NH, 1], F32),
            w2a=wt(f"fp{s}_w2a", [NH, NH]),
            w2b=wt(f"fp{s}_w2b", [NH, NH]),
            b2=wt(f"fp{s}_b2", [NH, 1], F32)))
    fin = dict(w1=wt("fin_w1", [NH, NH]), b1=wt("fin_b1", [NH, 1], F32),
               w2=wt("fin_w2", [NH, NOUT]), b2=wt("fin_b2", [NOUT, 1], F32),
               a2=wt("fin_a2", [NOUT, 1], F32), be2=wt("fin_be2", [NOUT, 1], F32))

    outT = nc.dram_tensor("outT", [NOUT, N0], F32, kind="ExternalOutput")

    with tile.TileContext(nc) as tc:
        with tc.tile_pool(name="const", bufs=1) as constp, \
             tc.tile_pool(name="feat", bufs=1) as featp, \
             tc.tile_pool(name="ztab", bufs=1) as ztabp, \
             tc.tile_pool(name="wpool", bufs=1) as wp, \
             tc.tile_pool(name="gath", bufs=4) as gathp, \
             tc.tile_pool(name="gfp", bufs=2) as gfpp, \
             tc.tile_pool(name="h1", bufs=2) as h1p, \
             tc.tile_pool(name="idx", bufs=4) as idxp, \
             tc.tile_pool(name="small", bufs=2) as smallp, \
             tc.tile_pool(name="dram", bufs=1, space="DRAM") as dramp, \
             tc.tile_pool(name="ps", bufs=1, space="PSUM") as ps, \
             tc.tile_pool(name="psu", bufs=6, space="PSUM") as psu:

            # ---- load inputs/weights to SBUF
            xT_t = constp.tile([NIN, N0], BF16)
            nc.sync.dma_start(xT_t[:], xT[:])
            posx_t = []
            for l in range(DEPTH):
                t = constp.tile([3, LEVELS[0] + 8], BF16, tag="posx")
                nc.vector.memset(t[:, LEVELS[l]:LEVELS[l] + 8], 0.0)
                nc.sync.dma_start(t[:, :LEVELS[l]], posT[l][:])
                posx_t.append(t)
            posq_t = []
            for l in range(DEPTH):
                t = constp.tile([3, LEVELS[1] // 2], BF16, tag="posq")
                nc.sync.dma_start(t[:, :LEVELS[l + 1] // 2], posQ[l][:])
                posq_t.append(t)

            def load_w(dram, shape, dt=BF16, tag=None):
                t = wp.tile(shape, dt, tag=tag or dram.name)
                nc.sync.dma_start(t[:], dram[:])
                return t

            lin_w_t = load_w(lin_w, [NIN, NH])
            lin_b_t = load_w(lin_b, [NH, 1], F32)
            saw_t = [{k: load_w(v, list(v.shape), v.dtype) for k, v in saw[l].items()}
                     for l in range(DEPTH)]
            fpw_t = [{k: load_w(v, list(v.shape), v.dtype) for k, v in fpw[s].items()}
                     for s in range(DEPTH)]
            fin_t = {k: load_w(v, list(v.shape), v.dtype) for k, v in fin.items()}

            ident = constp.tile([128, 128], BF16)
            from concourse.masks import make_identity
            make_identity(nc, ident[:])
            ones = constp.tile([1, 128], BF16, tag="ones")
            nc.vector.memset(ones[:], 1.0)

            # ---- lin_in: xb = relu(lin_w.T @ xT + lin_b) -> [128, 4097] bf16
            feats = []
            xb = featp.tile([NH, N0 + 1], BF16, tag="feat0")
            nc.vector.memset(xb[:, N0:], 0.0)
            for j in range(0, N0, 512):
                pt = psu.tile([NH, 512], F32, tag="mm")
                nc.tensor.matmul(pt[:], lin_w_t[:], xT_t[:, j:j + 512], start=True, stop=True)
                nc.scalar.activation(xb[:, j:j + 512], pt[:],
                                     mybir.ActivationFunctionType.Relu, bias=lin_b_t[:])
            feats.append(xb)

            # ---- SA levels
            for l in range(DEPTH):
                n, m = LEVELS[l], LEVELS[l + 1]
                w = saw_t[l]
                fprev, pprev = feats[l], posx_t[l]

                # shifted-by-one copies (aligned for PE moving operands)
                fshift = featp.tile([NH, LEVELS[0]], BF16, tag="fshift")
                pshift = featp.tile([3, LEVELS[0]], BF16, tag="pshift")
                nc.vector.tensor_copy(fshift[:, :n], fprev[:, 1:n + 1])
                nc.vector.tensor_copy(pshift[:, :n], pprev[:, 1:n + 1])
                # pair-stripe z table: 2n stripes x 384 elems.
                # stripe 2j (pair) lives at partition (2j)%128... NO: stripe id s
                # -> partition s%128, rank s//128. We instead INTERLEAVE ids so
                # content stays partition-aligned: stripe for pair-of-j has id
                # chosen by the HOST as j + n*(0) with ranks; see layout below.
                # Layout: ranks 0..n/128-1 hold PAIR stripes (rows j, j+1) with
                # id = j;   ranks n/128 .. 2n/128-1 hold SELF stripes (j, j)
                # with id = n + j.  Partition of id j is j%128 = row j's psum
                # partition for every write.
                ztab = ztabp.tile([128, (2 * LEVELS[0] // 128) * STRIPE], BF16, tag="ztab")
                for pc in range(n // 128):
                    # z rows for points j = pc*128 .. pc*128+127
                    pz = psu.tile([128, 512], F32, tag="mm")
                    nc.tensor.matmul(pz[:, :D], fprev[:, pc * 128:(pc + 1) * 128],
                                     w["w1f"][:], start=True, stop=False)
                    nc.tensor.matmul(pz[:, :D], pprev[:, pc * 128:(pc + 1) * 128],
                                     w["w1p"][:], start=False, stop=True)
                    # shifted: rows j+1 (reads one column beyond at the edge;
                    # feat/pos tables carry a zero pad column)
                    pzs = psu.tile([128, 512], F32, tag="mm")
                    nc.tensor.matmul(pzs[:, :D], fshift[:, pc * 128:(pc + 1) * 128],
                                     w["w1f"][:], start=True, stop=False)
                    nc.tensor.matmul(pzs[:, :D], pshift[:, pc * 128:(pc + 1) * 128],
                                     w["w1p"][:], start=False, stop=True)
                    o = pc * STRIPE
                    so = (n // 128) * STRIPE + pc * STRIPE
                    A = mybir.ActivationFunctionType.Identity
                    # pair stripe [zA_j | zA_j1 | zB_j zB_j1 pad] split across
                    # scalar+vector; self stripe [zA_j x2 | zB_j x2 pad] uses
                    # broadcast copies.
                    nc.scalar.activation(ztab[:, o:o + 128], pz[:, :128], A)
                    nc.scalar.activation(ztab[:, o + 256:o + 259], pz[:, 128:131], A)
                    nc.scalar.activation(ztab[:, o + 259:o + 262], pzs[:, 128:131], A)
                    nc.vector.tensor_copy(ztab[:, o + 128:o + 256], pzs[:, :128])
                    nc.vector.tensor_copy(
                        ztab[:, so:so + 256].rearrange("p (t c) -> p t c", t=2),
                        pz[:, :128][:, None, :].broadcast_to([128, 2, 128]))
                    nc.scalar.activation(
                        ztab[:, so + 256:so + 262].rearrange("p (t c) -> p t c", t=2),
                        pz[:, 128:131][:, None, :].broadcast_to([128, 2, 3]), A)

                # negc tables: negcA = W1pA^T q - b1A [128, mh], negcB6 likewise
                # [6, mh] (tail rows doubled).  relu(z+c) = max(z,-c)+c with
                # W2^T c hoisted into vnegb below.  All center-indexed tables
                # cover only THIS core's half (local columns).
                mh = m // 2
                q_t = posq_t[l]
                negcA = featp.tile([NH, LEVELS[1] // 2], BF16, tag="cA")
                negcB6 = featp.tile([6, LEVELS[1] // 2], BF16, tag="cB")
                for j in range(0, mh, 512):
                    e = min(512, mh - j)
                    pcA = psu.tile([NH, 512], F32, tag="mm")
                    nc.tensor.matmul(pcA[:, :e], w["w1p134"][:, :NH], q_t[:, j:j + e],
                                     start=True, stop=True)
                    nc.scalar.activation(negcA[:, j:j + e], pcA[:, :e],
                                         mybir.ActivationFunctionType.Identity,
                                         bias=w["b1An"][:])
                    pcB = ps.tile([6, 512], F32, tag="sm")
                    nc.tensor.matmul(pcB[:, :e], w["w1p134"][:, NH:], q_t[:, j:j + e],
                                     start=True, stop=True)
                    nc.scalar.activation(negcB6[:, j:j + e], pcB[:, :e],
                                         mybir.ActivationFunctionType.Identity,
                                         bias=w["b1B6n"][:])
                # vnegb = W2A^T negcA + W2t^T negcB3 - b2  (so that
                # xl = relu(hmaxU - vnegb))
                vnegb = featp.tile([NH, LEVELS[1] // 2], F32, tag="vnegb")
                for j in range(0, mh, 512):
                    e = min(512, mh - j)
                    pv = psu.tile([NH, 512], F32, tag="mm")
                    nc.tensor.matmul(pv[:, :e], w["w2A"][:], negcA[:, j:j + e],
                                     start=True, stop=False)
                    nc.tensor.matmul(pv[:, :e], w["w2t3"][:], negcB6[:3, j:j + e],
                                     start=False, stop=True)
                    nc.scalar.activation(vnegb[:, j:j + e], pv[:, :e],
                                         mybir.ActivationFunctionType.Identity,
                                         bias=w["negb2"][:])

                # main: per chunk gather pairs (k-major slots) + fused
                # t=max(z,-c) + layer2 + maxpool.  Chunks cover the core's half.
                hmaxU = featp.tile([NH, LEVELS[1] // 2], F32, tag="hmax")
                cpos = 0
                tab = chunk_tables[l]
                for gi in range(0, len(tab), 2):
                  grp = tab[gi:gi + 2]
                  Sg = sum(Cg * Kg for Cg, Kg, _ in grp)
                  gioff = grp[0][2]
                  gcols16 = (Sg + 15) // 16
                  it = idxp.tile([128, gcols16], I16, tag="idx")
                  nc.sync.dma_start(it[:], sa_idx[l][:, gioff // 16:gioff // 16 + gcols16])
                  g2 = gathp.tile([128, 3, Sg], BF16, tag="gsa")
                  nc.gpsimd.dma_gather(
                      out_ap=g2[:], in_ap=ztab[:], idxs_ap=it[:],
                      num_idxs=Sg, num_idxs_reg=Sg, elem_size=STRIPE,
                      transpose=True, single_packet=False,
                      sbuf_tokens_per_rank=128, sbuf_free_dim_per_rank=STRIPE * 2)
                  sbase = 0
                  for (C, Kp, ioff) in grp:
                    S = C * Kp
                    sb = sbase
                    sbase += S
                    te = h1p.tile([NH, S], BF16, tag="h1e")
                    to = h1p.tile([NH, S], BF16, tag="h1o")
                    tt = h1p.tile([8, S], BF16, tag="h1t")
                    for half, ht in ((0, te), (1, to)):
                        nc.vector.tensor_tensor(
                            out=ht[:].rearrange("p (k c) -> p k c", c=C),
                            in0=g2[:, half, sb:sb + S].rearrange("p (k c) -> p k c", c=C),
                            in1=negcA[:, cpos:cpos + C][:, None, :].broadcast_to([NH, Kp, C]),
                            op=mybir.AluOpType.max)
                    nc.vector.tensor_tensor(
                        out=tt[:6].rearrange("p (k c) -> p k c", c=C),
                        in0=g2[:6, 2, sb:sb + S].rearrange("p (k c) -> p k c", c=C),
                        in1=negcB6[:, cpos:cpos + C][:, None, :].broadcast_to([6, Kp, C]),
                        op=mybir.AluOpType.max)
                    # layer 2 + staged maxpool per psum-sized slice (512 = 32k x 16c)
                    n_sl = (S + 511) // 512
                    R = smallp.tile([NH, 4 * CC], F32, tag="rstage")
                    for si in range(n_sl):
                        s0 = si * 512
                        e = min(512, S - s0)
                        kk = e // C
                        pe = psu.tile([NH, 512], F32, tag="mm")
                        nc.tensor.matmul(pe[:, :e], w["w2A"][:], te[:, s0:s0 + e],
                                         start=True, stop=False)
                        nc.tensor.matmul(pe[:, :e], w["w2B"][:, :NH], tt[:6, s0:s0 + e],
                                         start=False, stop=True)
                        po = psu.tile([NH, 512], F32, tag="mm")
                        nc.tensor.matmul(po[:, :e], w["w2A"][:], to[:, s0:s0 + e],
                                         start=True, stop=False)
                        nc.tensor.matmul(po[:, :e], w["w2B"][:, NH:], tt[:6, s0:s0 + e],
                                         start=False, stop=True)
                        nc.vector.tensor_reduce(
                            out=R[:, 2 * si * CC:(2 * si + 1) * CC],
                            in_=pe[:, :e].rearrange("p (k c) -> p c k", c=C),
                            axis=mybir.AxisListType.X, op=mybir.AluOpType.max)
                        nc.vector.tensor_reduce(
                            out=R[:, (2 * si + 1) * CC:(2 * si + 2) * CC],
                            in_=po[:, :e].rearrange("p (k c) -> p c k", c=C),
                            axis=mybir.AxisListType.X, op=mybir.AluOpType.max)
                    nc.vector.tensor_reduce(
                        out=hmaxU[:, cpos:cpos + C],
                        in_=R[:, :2 * n_sl * CC].rearrange("p (s c) -> p c s", c=C),
                        axis=mybir.AxisListType.X, op=mybir.AluOpType.max)
                    cpos += C
                # x~_l(half) = relu(hmaxU - vnegb) = max(hmaxU, vnegb) - vnegb,
                # then pair-AllGather the two halves into the full xl.
                xl = featp.tile([NH, m + 1], BF16, tag=f"feat{l + 1}")
                nc.vector.memset(xl[:, m:], 0.0)
                xlh = h1p.tile([NH, LEVELS[1] // 2], BF16, tag="xlh")
                nc.vector.tensor_tensor(out=hmaxU[:, :mh], in0=hmaxU[:, :mh],
                                        in1=vnegb[:, :mh], op=mybir.AluOpType.max)
                nc.vector.tensor_tensor(out=xlh[:, :mh], in0=hmaxU[:, :mh],
                                        in1=vnegb[:, :mh], op=mybir.AluOpType.subtract)
                bin_ = dramp.tile([NH, mh], BF16, tag=f"agin{l}")
                bout = dramp.tile([2 * NH, mh], BF16, tag=f"agout{l}")
                nc.gpsimd.dma_start(bin_[:], xlh[:, :mh])
                nc.gpsimd.collective_compute(
                    "AllGather", mybir.AluOpType.bypass,
                    replica_groups=[[0, 1], [2, 3], [4, 5], [6, 7]],
                    ins=[bin_[:]], outs=[bout[:]])
                nc.sync.dma_start(xl[:, :mh], bout[:NH, :])
                nc.sync.dma_start(xl[:, mh:m], bout[NH:, :])
                feats.append(xl)

            # ---- FP stages (dense-S interpolation: y = xf @ S per fine chunk)
            xf = feats[DEPTH]
            for s in range(DEPTH):
                nf = nf_s[s]
                ncoarse = LEVELS[DEPTH - s]
                w = fpw_t[s]
                ftab = ztabp.tile([128, (LEVELS[1] // 128) * 128], BF16, tag="ftab")
                for r in range(ncoarse // 128):
                    ptr = ps.tile([128, 128], BF16, tag="tt")
                    nc.tensor.transpose(ptr[:], xf[:, r * 128:(r + 1) * 128], ident[:])
                    nc.vector.tensor_copy(ftab[:, r * 128:(r + 1) * 128], ptr[:])
                xfn = featp.tile([NH, nf + 1], BF16, tag=f"fpout{s}")
                nc.vector.memset(xfn[:, nf:], 0.0)
                skip = feats[DEPTH - 1 - s]
                for ci, (base, W, coloff) in enumerate(fp_tabs[s]):
                    j = ci * CHF
                    py = psu.tile([NH, 512], F32, tag="mm")
                    nb = W // 128
                    for r in range(nb):
                        St = gfpp.tile([128, CHF], BF16, tag="fpS")
                        nc.sync.dma_start(St[:], fp_S[s][:, coloff + r * CHF:coloff + (r + 1) * CHF])
                        nc.tensor.matmul(py[:], ftab[:, base + r * 128:base + (r + 1) * 128],
                                         St[:],
                                         start=(r == 0), stop=(r == nb - 1))
                    y = h1p.tile([NH, 512], BF16, tag="y")
                    nc.scalar.activation(y[:], py[:], mybir.ActivationFunctionType.Identity)
                    h1a = h1p.tile([NH, 512], BF16, tag="fph1a")
                    h1b = h1p.tile([NH, 512], BF16, tag="fph1b")
                    pa = psu.tile([NH, 512], F32, tag="mm")
                    nc.tensor.matmul(pa[:], w["w1y"][:, :NH], y[:], start=True, stop=False)
                    nc.tensor.matmul(pa[:], w["w1s"][:, :NH], skip[:, j:j + 512],
                                     start=False, stop=True)
                    nc.scalar.activation(h1a[:], pa[:],
                                         mybir.ActivationFunctionType.Relu, bias=w["b1a"][:])
                    pb = psu.tile([NH, 512], F32, tag="mm")
                    nc.tensor.matmul(pb[:], w["w1y"][:, NH:], y[:], start=True, stop=False)
                    nc.tensor.matmul(pb[:], w["w1s"][:, NH:], skip[:, j:j + 512],
                                     start=False, stop=True)
                    nc.scalar.activation(h1b[:], pb[:],
                                         mybir.ActivationFunctionType.Relu, bias=w["b1b"][:])
                    pc2 = psu.tile([NH, 512], F32, tag="mm")
                    nc.tensor.matmul(pc2[:], w["w2a"][:], h1a[:], start=True, stop=False)
                    nc.tensor.matmul(pc2[:], w["w2b"][:], h1b[:], start=False, stop=True)
                    nc.scalar.activation(xfn[:, j:j + 512], pc2[:],
                                         mybir.ActivationFunctionType.Relu, bias=w["b2"][:])
                xf = xfn

            # ---- final head
            for j in range(0, N0, 256):
                ph = psu.tile([NH, 512], F32, tag="mm")
                nc.tensor.matmul(ph[:, :256], fin_t["w1"][:], xf[:, j:j + 256], start=True, stop=True)
                hh = h1p.tile([NH, 256], BF16, tag="finh")
                nc.scalar.activation(hh[:], ph[:, :256], mybir.ActivationFunctionType.Relu,
                                     bias=fin_t["b1"][:])
                po = ps.tile([NOUT, 256], F32, tag="sm")
                nc.tensor.matmul(po[:], fin_t["w2"][:], hh[:], start=True, stop=True)
                oo = h1p.tile([NOUT, 256], F32, tag="fino")
                nc.scalar.activation(oo[:], po[:], mybir.ActivationFunctionType.Relu,
                                     bias=fin_t["b2"][:])
                nc.vector.tensor_scalar(out=oo[:], in0=oo[:],
                                        scalar1=fin_t["a2"][:], scalar2=fin_t["be2"][:],
                                        op0=mybir.AluOpType.mult, op1=mybir.AluOpType.add)
                nc.sync.dma_start(outT[:, j:j + 256], oo[:])

    nc.compile()
    return nc


# ---------------------------------------------------------------- packaging

def make_core_inputs(xc, cloud, W, chunk_tables, fp_tabs, parity):
    """Per-core inputs for one cloud.  xc: [N0, NIN] in ORIGINAL row order.
    All level orders are the cloud's morton orders; stripe id of pair (j,j+1)
    at level l is j, self stripe (j,j) is n + j.  parity selects which half
    of each level's centers this core computes."""
    d = {}
    perm0 = cloud["perm"][0]
    d["xT"] = np.ascontiguousarray(xc[perm0].T).astype(bf16)
    for l in range(DEPTH):
        d[f"posT{l}"] = np.ascontiguousarray(cloud["pos"][l].T).astype(bf16)
    for l in range(DEPTH):
        mh = LEVELS[l + 1] // 2
        d[f"posQ{l}"] = np.ascontiguousarray(
            cloud["pos"][l + 1][parity * mh:(parity + 1) * mh].T).astype(bf16)
    for l in range(DEPTH):
        n = LEVELS[l]
        mh = LEVELS[l + 1] // 2
        wins = cloud["sa_wins"][l]
        flat = []
        cpos = parity * mh
        for (C, Kp, _off) in chunk_tables[l]:
            arr = np.empty((C, Kp), np.int64)
            for k in range(C):
                w = wins[cpos + k]
                # stripe id: even ids are pairs (id 2j -> stripe j at rank j//128),
                # odd are self (2j+1 -> stripe n + j).  Convert:
                sid = np.where(w % 2 == 0, w // 2, n + (w - 1) // 2)
                pad = np.full(Kp - len(sid), sid[0] if len(sid) else n, np.int64)
                arr[k] = np.concatenate([sid, pad])
            flat.append(arr.T.reshape(-1))   # k-major slots: (k, c)
            cpos += C
        flat = np.concatenate(flat)
        d[f"sa_idx{l}"] = wrap_idx(flat)
    for s in range(DEPTH):
        nf = LEVELS[DEPTH - 1 - s]
        idxr = cloud["fp_idx"][s].reshape(nf, KNN)
        wr = cloud["fp_w"][s].reshape(nf, KNN)
        tab = fp_tabs[s]
        ncols = tab[-1][2] + (tab[-1][1] // 128) * CHF
        S = np.zeros((128, ncols), np.float32)
        for ci, (base, Wd, coloff) in enumerate(tab):
            j = ci * CHF
            for k in range(KNN):
                rel = idxr[j:j + CHF, k] - base          # in [0, Wd)
                part = rel % 128
                blk = rel // 128
                cols = coloff + blk * CHF + np.arange(CHF)
                np.add.at(S, (part, cols), wr[j:j + CHF, k])
        d[f"fp_S{s}"] = S.astype(bf16)
    d["lin_w"] = W["lin_w"].astype(bf16)
    d["lin_b"] = W["lin_b"].reshape(NH, 1).astype(np.float32)
    for l in range(DEPTH):
        Wl = W[f"sa{l}"]
        d[f"sa{l}_w1f"] = Wl["w1_feat"].astype(bf16)
        d[f"sa{l}_w1p"] = Wl["w1_pos"].astype(bf16)
        wp = Wl["w1_pos"]                     # [3, 131] POSITIVE (for negc)
        wp134 = np.concatenate([wp[:, :NH], wp[:, NH:], wp[:, NH:]], axis=1)
        d[f"sa{l}_w1p134"] = wp134.astype(bf16)  # [3, 134]
        d[f"sa{l}_b1An"] = (-Wl["b1"][:NH]).reshape(NH, 1).astype(np.float32)
        d[f"sa{l}_b1B6n"] = (-np.concatenate([Wl["b1"][NH:], Wl["b1"][NH:]])).reshape(6, 1).astype(np.float32)
        d[f"sa{l}_w2A"] = Wl["w2"][:NH, :].astype(bf16)
        w2b6 = np.zeros((6, 2 * NH), np.float32)
        w2b6[0:3, :NH] = Wl["w2"][NH:, :]      # even bank: rows 0..2
        w2b6[3:6, NH:] = Wl["w2"][NH:, :]      # odd bank: rows 3..5
        d[f"sa{l}_w2B"] = w2b6.astype(bf16)
        d[f"sa{l}_w2t3"] = Wl["w2"][NH:, :].astype(bf16)
        d[f"sa{l}_negb2"] = (-Wl["b2"]).reshape(NH, 1).astype(np.float32)
    for s in range(DEPTH):
        Ws = W[f"fp{s}"]
        d[f"fp{s}_w1y"] = Ws["w1y"].astype(bf16)
        d[f"fp{s}_w1s"] = Ws["w1s"].astype(bf16)
        d[f"fp{s}_b1a"] = Ws["b1"][:NH].reshape(NH, 1).astype(np.float32)
        d[f"fp{s}_b1b"] = Ws["b1"][NH:].reshape(NH, 1).astype(np.float32)
        d[f"fp{s}_w2a"] = Ws["w2"][:NH, :].astype(bf16)
        d[f"fp{s}_w2b"] = Ws["w2"][NH:, :].astype(bf16)
        d[f"fp{s}_b2"] = Ws["b2"].reshape(NH, 1).astype(np.float32)
    Wf = W["final"]
    d["fin_w1"] = Wf["w1"].astype(bf16)
    d["fin_b1"] = Wf["b1"].reshape(NH, 1).astype(np.float32)
    d["fin_w2"] = Wf["w2"].astype(bf16)
    d["fin_b2"] = Wf["b2"].reshape(NOUT, 1).astype(np.float32)
    d["fin_a2"] = Wf["a2"].reshape(NOUT, 1).astype(np.float32)
    d["fin_be2"] = Wf["be2"].reshape(NOUT, 1).astype(np.float32)
    return d


_KERNEL_CACHE = {}


def kernel(trace=False, **inputs):
    clouds = host_indices(np.asarray(inputs["pos"], np.float32))
    W = fold_weights(inputs)
    chunk_tables = make_plan(clouds)
    fp_tabs = make_fp_plan(clouds)
    key = (tuple(tuple(t) for tab in chunk_tables for t in tab),
           tuple(tuple(t) for tab in fp_tabs for t in tab))
    if key not in _KERNEL_CACHE:
        _KERNEL_CACHE.clear()
        _KERNEL_CACHE[key] = build_kernel(chunk_tables, fp_tabs)
    nc = _KERNEL_CACHE[key]
    xr = np.asarray(inputs["x"], np.float32).reshape(B, N0, NIN)
    in_maps = []
    for core in range(8):
        b = core // 2
        in_maps.append(make_core_inputs(xr[b], clouds[b], W, chunk_tables,
                                        fp_tabs, core % 2))
    res = run_bass_kernel_spmd(nc, in_maps, core_ids=list(range(8)), trace=trace)
    outs = []
    for b in range(B):
        ot = np.ascontiguousarray(res.results[2 * b]["outT"].T)   # [N0, 2] morton
        o = np.empty_like(ot)
        o[clouds[b]["perm"][0]] = ot                               # undo morton
        outs.append(o)
    out = np.concatenate(outs, 0).astype(np.float32)
    if trace:
        kernel.last_exec_time_ns = res.exec_time_ns
        kernel.last_trace = res.instructions_and_trace
    return out


kernel.last_exec_time_ns = None
kernel.last_trace = None

